# revision 4
# baseline (speedup 1.0000x reference)
"""DeepSpeed-style MLP block (pre-LN residual add + LN + GEMM+GELU + GEMM +
residual) for Trainium2, data-parallel over tokens across 8 NeuronCores.

Both GEMMs run in fp8 e4m3 with perf_mode=DoubleRow (2 fp8 weights per PE
cell, K=256 per matmul, ~216 ns per 128x256x512 DR matmul = the per-row PE
roofline). Weights are scaled x1024 (clipped to +-240 - TRN e4m3 overflows to
inf) and the LN output x16, folded back out through the activation scale and
the final evict. rel err ~1.79e-2 vs the fp32 reference (gate 2e-2).

Per-core schedule (tokens sharded 8 x 4096, processed in 512-token blocks):
  r    = input(+bias, host) + residual     (accumulating SWDGE DMA; block 0:
                                            HWDGE loads on sync+scalar queues
                                            + gpsimd tensor_tensor adds)
  x0   = 16 * LN(r)                        (bn_stats/aggr + ACT Rsqrt; LN
                                            affine folded into W1/b1 on host)
  xT   = DMA-xbar transpose of x0 (SBUF->SBUF, one DMA per 128-token group)
         + one DVE cast per group into the fp8 [128, KH, TB] layout
         (block 0 uses PE transposes while the PE is otherwise idle)
  hT   = gelu_tanh(W1'-pairs.T @ xT + b1)  (DR matmuls + ACT, fp8 out)
  out  = hT-pairs.T @ W2 / 1024 + (r + output_b)

GEMM1 is front-loaded 2 m-pairs : 1 GEMM2-k-pair so GEMM1 finishes at ~2/3 of
the block; the tail runs g-major: per 128-token group the remaining n0
k-pairs, the n0 evict, all 16 n1 k-pairs (accumulated in a freed GEMM1 PSUM
bank), and the n1 evict - no PE stalls at block boundaries. ~20 garbage
ident-matmuls at t~8us flip the HAM clock gate to 2.4 GHz before real work;
weights load as 4+2 wide DMAs spread across the sync/scalar/gpsimd queues in
parallel with the block-0 token loads. Block 0 runs GEMM1 in two 256-token
half phases so the PE starts ~19.5us in.
"""

import sys

sys.path.insert(0, "/opt/trn_rl_repo")

import numpy as np
import ml_dtypes

import concourse.bass as bass
import concourse.mybir as mybir
import concourse.tile as tile
from concourse.masks import make_identity
from concourse.bass_utils import run_bass_kernel_spmd

AFT = mybir.ActivationFunctionType
ALU = mybir.AluOpType
DR = mybir.MatmulPerfMode.DoubleRow
FP32 = mybir.dt.float32
BF16 = mybir.dt.bfloat16
FP8 = mybir.dt.float8e4

N_CORES = 8
B, S, H, I = 4, 8192, 1024, 4096
NTOK = B * S              # 32768 tokens total
T = NTOK // N_CORES       # 4096 tokens per core
TB = 512                  # tokens per block (moving free dim)
G = TB // 128             # 4 token sub-tiles per block
KH = H // 128             # 8 contraction chunks for GEMM1
MI = I // 128             # 32 I-chunks (GEMM1 out / GEMM2 contraction)
EPS = 1e-5

S_X = 16.0                # fp8 scale on the LN output x
S_W = 1024.0              # fp8 scale on W1 and W2
C1 = 1.0 / (S_X * S_W)    # GEMM1 psum -> z

# DMA-xbar transpose row map: True -> transposed row h lands at
# (partition p = h // KH, sub-dim k = h % KH); False -> (p = h % 128,
# k = h // 128). w1's load pattern and the block-0 PE transposes follow.
# (HW-measured: the 3D-out xbar produces the natural k*128+p layout.)
XBAR_PK = False
USE_XBAR = True           # False: PE transposes everywhere (fallback)


def _split_multiwait_instructions(nc):
    """This walrus build accepts only ONE sync-wait command per instruction.
    Move extra waits onto fresh same-engine NOPs placed just before the
    offending instruction."""
    n_split = 0
    for f in nc.m.functions:
        for bb in f.blocks:
            insts = list(bb.instructions)
            new = []
            changed = False
            for inst in insts:
                si = inst.sync_info
                if si is not None and si.on_wait and len(si.on_wait) > 1:
                    waits = list(si.on_wait)
                    for w in waits[:-1]:
                        nop = mybir.InstNoOp(name=nc.get_next_instruction_name())
                        nop.engine = inst.engine
                        nop.sync_info = mybir.SyncInfo(on_wait=[w], on_update=[])
                        new.append(nop)
                        n_split += 1
                    si.on_wait = waits[-1:]
                    changed = True
                new.append(inst)
            if changed:
                bb.instructions = new
    return n_split


def _bcast_ap(ap, p=128):
    """AP view of a DRAM vector broadcast across p partitions."""
    return bass.AP(tensor=ap.tensor, offset=ap.offset, ap=[[0, p]] + list(ap.ap))


def _build(n_blocks=T // TB):
    nc = bass.Bass("TRN2")
    t_rows = n_blocks * TB
    xin = nc.declare_dram_parameter("xin", [t_rows, H], FP32, isOutput=False)
    res = nc.declare_dram_parameter("res", [t_rows, H], FP32, isOutput=False)
    w1 = nc.declare_dram_parameter("w1", [H, I], FP8, isOutput=False)
    w2 = nc.declare_dram_parameter("w2", [I, H], FP8, isOutput=False)
    b1c = nc.declare_dram_parameter("b1c", [128, MI], FP32, isOutput=False)
    b2v = nc.declare_dram_parameter("b2v", [H], FP32, isOutput=False)
    out = nc.declare_dram_parameter("out", [t_rows, H], FP32, isOutput=True)

    with tile.TileContext(nc) as tc:
        with (
            tc.tile_pool(name="const", bufs=1) as const,
            tc.tile_pool(name="ing", bufs=4) as ing,
            tc.tile_pool(name="tmpg", bufs=4) as tmpg,
            tc.tile_pool(name="blk1", bufs=1) as blk1,
            tc.tile_pool(name="blk2", bufs=2) as blk2,
            tc.tile_pool(name="hTp", bufs=2) as hTp,
            tc.tile_pool(name="outp", bufs=4) as outp,
            tc.tile_pool(name="statp", bufs=2) as statp,
            tc.tile_pool(name="ps1", bufs=4, space="PSUM") as ps1,
            tc.tile_pool(name="ps2", bufs=4, space="PSUM") as ps2,
        ):
            # ---- consts; everything off the sync/scalar queues so the
            # block-0 token loads go out first ----
            ident = const.tile([128, 128], BF16)
            make_identity(nc, ident)                      # gpsimd
            warm_rhs = const.tile([128, 512], BF16)
            nc.vector.memset(warm_rhs, 0.0)
            eps_t = const.tile([128, 1], FP32)
            nc.vector.memset(eps_t, EPS / (S_X * S_X))
            b2_bc = const.tile([128, H], FP32)
            nc.gpsimd.dma_start(out=b2_bc, in_=_bcast_ap(b2v[:]))
            b1_sb = const.tile([128, MI], FP32)
            nc.gpsimd.dma_start(out=b1_sb, in_=b1c[:, :])
            w1_sb = const.tile([128, KH, I], FP8, name="w1_sb")
            w2_sb = const.tile([128, MI, H], FP8, name="w2_sb")

            # ---- HAM warmup: garbage matmuls flip the PE clock gate to
            # 2.4 GHz (~3.4us busy) and keep it there until real work ----
            warm_ps = ps1.tile([128, 512], FP32, name="warm", tag="p1")
            for _ in range(20):
                nc.tensor.matmul(warm_ps, lhsT=ident, rhs=warm_rhs, start=True, stop=True)

            # ---- LN pieces ----
            def ln_alloc(tb):
                return {
                    "x0": blk1.tile([128, G, H], BF16, name=f"x0_{tb}", tag="x0"),
                    "xT8": blk1.tile([128, KH, TB], FP8, name=f"xT8_{tb}", tag="xT8"),
                    "xTb": (
                        blk1.tile([128, KH, TB], BF16, name=f"xTb_{tb}", tag="xTb")
                        if USE_XBAR
                        else None
                    ),
                    "r32": blk2.tile([128, G, H], FP32, name=f"r32_{tb}", tag="r32"),
                    "mvb": statp.tile([128, G, 2], FP32, name=f"mvb_{tb}", tag="mvb"),
                    "rstd": statp.tile([128, G], FP32, name=f"rsd_{tb}", tag="rstd"),
                    "tmp": [None] * G,
                }

            def ln_stats(tb, g, st):
                stats = statp.tile([128, 2, 6], FP32, name=f"st_{tb}_{g}", tag="stats")
                tmp_r = st["tmp"][g].rearrange("p (s d) -> p s d", s=2)
                for s_ in range(2):
                    nc.vector.bn_stats(out=stats[:, s_, :], in_=tmp_r[:, s_, :])
                nc.vector.bn_aggr(out=st["mvb"][:, g, :], in_=stats)

            def ln_chunk(tb, g, st):
                """Steady-state: HWDGE xin load + accumulating SWDGE res add."""
                t0 = tb * TB
                ra, rb = t0 + g * 128, t0 + (g + 1) * 128
                tmp = tmpg.tile([128, H], FP32, name=f"tmp_{tb}_{g}", tag="tmp")
                nc.sync.dma_start(out=tmp, in_=xin[ra:rb, :])
                nc.gpsimd.dma_start(out=tmp, in_=res[ra:rb, :], accum_op=ALU.add)
                st["tmp"][g] = tmp
                ln_stats(tb, g, st)

            def ln_rstd(tb, st, gs):
                # sqrt((var+eps)/S_X^2) then reciprocal -> S_X * rsqrt(var+eps)
                gsl = slice(gs[0], gs[-1] + 1)
                nc.scalar.activation(
                    out=st["rstd"][:, gsl], in_=st["mvb"][:, gsl, 1], func=AFT.Sqrt,
                    bias=eps_t, scale=1.0 / (S_X * S_X),
                )
                nc.vector.reciprocal(out=st["rstd"][:, gsl], in_=st["rstd"][:, gsl])

            def ln_x0(tb, g, st):
                nc.vector.tensor_scalar(
                    out=st["x0"][:, g, :],
                    in0=st["tmp"][g],
                    scalar1=st["mvb"][:, g, 0:1],
                    scalar2=st["rstd"][:, g : g + 1],
                    op0=ALU.subtract,
                    op1=ALU.mult,
                )

            def ln_r32(tb, g, st):
                nc.vector.tensor_add(out=st["r32"][:, g, :], in0=st["tmp"][g], in1=b2_bc)

            def ln_finish(tb, st):
                """Steady-state finish: r32 first (no deps, fills the DVE while
                the ACT Rsqrt gates the x0 writes), then the 4 x0 writes."""
                for g in range(G):
                    ln_r32(tb, g, st)
                ln_rstd(tb, st, list(range(G)))
                for g in range(G):
                    ln_x0(tb, g, st)

            def ln_xbar(tb, g, st):
                """Steady-state transpose: one SBUF->SBUF xbar DMA + DVE cast."""
                sl = slice(g * 128, (g + 1) * 128)
                if USE_XBAR:
                    nc.sync.dma_start(
                        out=st["xTb"][:, :, sl], in_=st["x0"][:, g, :], transpose=True
                    )
                    nc.vector.tensor_copy(out=st["xT8"][:, :, sl], in_=st["xTb"][:, :, sl])
                else:
                    ptg = ps2.tile([128, KH, 128], BF16, name=f"pt_{tb}_{g}", tag="p2")
                    for k in range(KH):
                        nc.tensor.transpose(ptg[:, k, :], _tsrc(st, g, k), ident)
                    nc.vector.tensor_copy(out=st["xT8"][:, :, sl], in_=ptg)

            def _tsrc(st, g, k):
                """PE-transpose source for chunk k, matching the xbar row map."""
                if XBAR_PK and USE_XBAR:
                    return st["x0"][:, g, :].rearrange("p (x e) -> p e x", e=KH)[:, k, :]
                return st["x0"][:, g, k * 128 : (k + 1) * 128]

            # ---- GEMM pieces ----
            def g1_chunk(tb, m, tiles, lo=0, hi=TB):
                p1 = ps1.tile([128, TB], FP32, name=f"p1_{tb}_{m}_{lo}", tag="p1")
                for k in range(KH // 2):
                    nc.tensor.matmul(
                        p1[:, lo:hi],
                        lhsT=w1_sb[:, 2 * k : 2 * k + 2, m * 128 : (m + 1) * 128],
                        rhs=tiles["xT8"][:, 2 * k : 2 * k + 2, lo:hi],
                        start=(k == 0),
                        stop=(k == KH // 2 - 1),
                        perf_mode=DR,
                    )
                nc.scalar.activation(
                    out=tiles["hT"][:, m, lo:hi], in_=p1[:, lo:hi],
                    func=AFT.Gelu_apprx_tanh, bias=b1_sb[:, m : m + 1], scale=C1,
                )

            def g2_mm(tiles, n, k, ps, g, start, stop):
                nc.tensor.matmul(
                    ps,
                    lhsT=tiles["hT"][:, 2 * k : 2 * k + 2, g * 128 : (g + 1) * 128],
                    rhs=w2_sb[:, 2 * k : 2 * k + 2, n * 512 : (n + 1) * 512],
                    start=start,
                    stop=stop,
                    perf_mode=DR,
                )

            def evict(tb, n, g, ps, tiles):
                t0 = tb * TB
                o = outp.tile([128, 512], FP32, name=f"o_{tb}_{n}_{g}", tag="o")
                nc.vector.scalar_tensor_tensor(
                    out=o,
                    in0=ps,
                    scalar=1.0 / S_W,
                    in1=tiles["r32"][:, g, n * 512 : (n + 1) * 512],
                    op0=ALU.mult,
                    op1=ALU.add,
                )
                eng = nc.sync if tb == n_blocks - 1 else nc.gpsimd
                eng.dma_start(
                    out=out[t0 + g * 128 : t0 + (g + 1) * 128, n * 512 : (n + 1) * 512],
                    in_=o,
                )

            def block_tail(tb, tiles, p2s, st_next, pending):
                """g-major: per token group, prep next block's transpose, the
                remaining n0 k-pairs, n0 evict, full n1 accumulation in a
                freed GEMM1 PSUM bank, n1 evict."""
                for g in range(G):
                    if st_next is not None:
                        ln_xbar(tb + 1, g, st_next)
                    if p2s[g] is None:
                        p2s[g] = ps2.tile(
                            [128, 512], FP32, name=f"p20_{tb}_{g}", tag="p2"
                        )
                    for k in pending:
                        g2_mm(tiles, 0, k, p2s[g], g, start=(k == 0), stop=(k == MI // 2 - 1))
                    evict(tb, 0, g, p2s[g], tiles)
                    p1x = ps1.tile([128, TB], FP32, name=f"p21_{tb}_{g}", tag="p1")
                    for k in range(MI // 2):
                        g2_mm(tiles, 1, k, p1x, g, start=(k == 0), stop=(k == MI // 2 - 1))
                    evict(tb, 1, g, p1x, tiles)

            # ================= block 0 =================
            # All block-0 + weight DMA issues up front, spread across queues:
            # sync: xin g0-g3, w1 q0, q1 | scalar: res g0-g3, w1 q2, q3 |
            # gpsimd: pre-LN adds interleaved with the two w2 halves.
            w1_re = "(p k) h -> p k h" if (XBAR_PK and USE_XBAR) else "(k p) h -> p k h"
            st0 = ln_alloc(0)
            for g in range(G):
                tmp = tmpg.tile([128, H], FP32, name=f"tmp_0_{g}", tag="tmp")
                nc.sync.dma_start(out=tmp, in_=xin[g * 128 : (g + 1) * 128, :])
                st0["tmp"][g] = tmp
            res0 = []
            for g in range(G):
                rg = ing.tile([128, H], FP32, name=f"res_0_{g}", tag="res")
                nc.scalar.dma_start(out=rg, in_=res[g * 128 : (g + 1) * 128, :])
                res0.append(rg)
            for qi, eng in ((0, nc.sync), (1, nc.sync), (2, nc.scalar), (3, nc.scalar)):
                q = slice(qi * (I // 4), (qi + 1) * (I // 4))
                eng.dma_start(
                    out=w1_sb[:, :, q], in_=w1[:, q].rearrange(w1_re, p=128)
                )
            for g in range(G):
                nc.gpsimd.tensor_tensor(
                    out=st0["tmp"][g], in0=st0["tmp"][g], in1=res0[g], op=ALU.add
                )
                if g == 1 or g == 3:
                    hs = slice((g // 2) * 16, (g // 2) * 16 + 16)
                    nc.gpsimd.dma_start(
                        out=w2_sb[:, hs, :],
                        in_=w2[(g // 2) * 2048 : (g // 2) * 2048 + 2048, :].rearrange(
                            "(k p) h -> p k h", p=128
                        ),
                    )

            # Per-g LN finish: PE transposes (PE is idle; also keeps HAM hot),
            # casts on ACT so the DVE only carries stats + x0.
            ptg0 = [None] * G

            def b0_trans(g):
                ptg = ps2.tile([128, KH, 128], BF16, name=f"pt0_{g}", tag="p2")
                for k in range(KH):
                    nc.tensor.transpose(ptg[:, k, :], _tsrc(st0, g, k), ident)
                ptg0[g] = ptg

            def b0_cast(g):
                nc.scalar.activation(
                    out=st0["xT8"][:, :, g * 128 : (g + 1) * 128], in_=ptg0[g],
                    func=AFT.Copy,
                )

            ln_stats(0, 0, st0)
            ln_stats(0, 1, st0)
            ln_rstd(0, st0, [0])
            ln_x0(0, 0, st0)
            b0_trans(0)
            b0_cast(0)
            ln_stats(0, 2, st0)
            ln_rstd(0, st0, [1])
            ln_x0(0, 1, st0)
            b0_trans(1)
            b0_cast(1)
            ln_stats(0, 3, st0)
            ln_rstd(0, st0, [2])
            ln_x0(0, 2, st0)
            b0_trans(2)
            b0_cast(2)
            ln_rstd(0, st0, [3])
            ln_x0(0, 3, st0)
            b0_trans(3)
            b0_cast(3)
            for g in range(G):
                ln_r32(0, g, st0)

            # Phase A: GEMM1 on the first 256 tokens (g0+g1 only).
            tiles = {
                "xT8": st0["xT8"],
                "r32": st0["r32"],
                "hT": hTp.tile([128, MI, TB], FP8, name="hT_0", tag="hT"),
            }
            for m in range(MI):
                g1_chunk(0, m, tiles, 0, 256)
            # Phase B: GEMM1 on the second 256 tokens, interleaved with the
            # n0 k-pairs for g0/g1 (their hT halves are fully ready) and
            # block-1 LN chunks.
            st_next = ln_alloc(1) if n_blocks > 1 else None
            p2s = [None] * G
            for m in range(MI):
                g1_chunk(0, m, tiles, 256, 512)
                if m % 2 == 1:
                    k = m // 2
                    for g in (0, 1):
                        if k == 0:
                            p2s[g] = ps2.tile(
                                [128, 512], FP32, name=f"p20_0_{g}", tag="p2"
                            )
                        g2_mm(tiles, 0, k, p2s[g], g, start=(k == 0), stop=(k == MI // 2 - 1))
                if st_next is not None and m % 8 == 5:
                    ln_chunk(1, m // 8, st_next)
            if st_next is not None:
                ln_finish(1, st_next)
            # Phase C: g-major tails (g0/g1 n0 already fully accumulated).
            pending0 = {0: [], 1: [], 2: list(range(MI // 2)), 3: list(range(MI // 2))}

            def block0_tail():
                for g in range(G):
                    if st_next is not None:
                        ln_xbar(1, g, st_next)
                    if p2s[g] is None:
                        p2s[g] = ps2.tile([128, 512], FP32, name=f"p20_0_{g}", tag="p2")
                    for k in pending0[g]:
                        g2_mm(tiles, 0, k, p2s[g], g, start=(k == 0), stop=(k == MI // 2 - 1))
                    evict(0, 0, g, p2s[g], tiles)
                    p1x = ps1.tile([128, TB], FP32, name=f"p21_0_{g}", tag="p1")
                    for k in range(MI // 2):
                        g2_mm(tiles, 1, k, p1x, g, start=(k == 0), stop=(k == MI // 2 - 1))
                    evict(0, 1, g, p1x, tiles)

            block0_tail()
            if st_next is not None:
                tiles = {"xT8": st_next["xT8"], "r32": st_next["r32"]}

            # ================= steady blocks =================
            for tb in range(1, n_blocks):
                tiles["hT"] = hTp.tile([128, MI, TB], FP8, name=f"hT_{tb}", tag="hT")
                st_next = ln_alloc(tb + 1) if tb + 1 < n_blocks else None
                p2s = [
                    ps2.tile([128, 512], FP32, name=f"p20_{tb}_{g}", tag="p2")
                    for g in range(G)
                ]
                for it in range(8):
                    for c in range(4):
                        g1_chunk(tb, 4 * it + c, tiles)
                    if it >= 1:
                        for g in range(G):
                            g2_mm(tiles, 0, it - 1, p2s[g], g, start=(it == 1), stop=False)
                    if st_next is not None and it % 2 == 1:
                        ln_chunk(tb + 1, (it - 1) // 2, st_next)
                if st_next is not None:
                    ln_finish(tb + 1, st_next)
                block_tail(tb, tiles, p2s, st_next, list(range(7, MI // 2)))
                if st_next is not None:
                    tiles = {"xT8": st_next["xT8"], "r32": st_next["r32"]}

    return nc


def _prep_inputs(input, residual, bias, attn_nw, attn_nb, inter_w, inter_b, output_w, output_b):
    """Host-side preprocessing: fold bias into the input stream and the LN
    affine into W1/b1, scale + cast weights to fp8 e4m3 (clip to +-240: TRN
    e4m3 overflows to inf), shard tokens."""
    f8 = ml_dtypes.float8_e4m3
    biasf = np.asarray(bias, np.float32)
    x2 = np.ascontiguousarray(
        np.asarray(input, np.float32).reshape(NTOK, H) + biasf
    )
    r2 = np.ascontiguousarray(np.asarray(residual, np.float32).reshape(NTOK, H))
    gamma = np.asarray(attn_nw, np.float64)
    beta = np.asarray(attn_nb, np.float64)
    w1f = np.asarray(inter_w, np.float64)
    w2f = np.asarray(output_w, np.float64)
    w1p = gamma[:, None] * w1f
    w1b = np.ascontiguousarray(
        np.clip(w1p * S_W, -240, 240).astype(np.float32).astype(f8)
    )
    b1p = (np.asarray(inter_b, np.float64) + beta @ w1f).astype(np.float32)
    b1c = np.ascontiguousarray(b1p.reshape(MI, 128).T)
    w2b = np.ascontiguousarray(
        np.clip(w2f * S_W, -240, 240).astype(np.float32).astype(f8)
    )
    b2f = np.asarray(output_b, np.float32)

    in_maps = []
    for c in range(N_CORES):
        sl = slice(c * T, (c + 1) * T)
        im = {
            "xin": x2[sl],
            "res": r2[sl],
            "w1": w1b,
            "w2": w2b,
            "b1c": b1c,
            "b2v": b2f,
        }
        in_maps.append(im)
    return in_maps


def _run(inputs, trace=False, **kwargs):
    in_maps = _prep_inputs(
        inputs["input"],
        inputs["residual"],
        inputs["bias"],
        inputs["attn_nw"],
        inputs["attn_nb"],
        inputs["inter_w"],
        inputs["inter_b"],
        inputs["output_w"],
        inputs["output_b"],
    )
    nc = _build()
    _split_multiwait_instructions(nc)
    r = run_bass_kernel_spmd(nc, in_maps, list(range(N_CORES)), trace=trace, **kwargs)
    outs = [r.results[c]["out"] for c in range(N_CORES)]
    full = np.concatenate(outs, axis=0).reshape(B, S, H).astype(np.float32)
    return full, r


def kernel(**inputs):
    out, _ = _run(inputs, trace=False)
    return out


if __name__ == "__main__":
    nc = _build(2)
    print("built 2-block variant ok:", len(nc.m.functions[0].blocks))
